# revision 7
# baseline (speedup 1.0000x reference)
"""Multi-head attention (B=2, S=2048, H=1024, 16 heads x 64) on 8 trn2 cores.

Sharding: core c handles batch b=c//4 and the 4 heads [4*(c%4) .. 4*(c%4)+3]
(tensor-parallel over the hd=256 column slice of Wq/Wk/Wv and the matching
row slice of Wo).  Each core computes a rank-256 partial of the output
projection for its batch; the host sums the 4 partials per batch and adds bo.

Device kernel (per core, all in bf16 matmuls with fp32 PSUM accumulate):
  QT[hd,s] = Wq_c^T X_b^T   (lhsT=Wq nat. layout, rhs=X^T prepped on host)
  KT[hd,s], V[s,hd] similarly.
  Per head pair (2 heads packed in the 128-partition dim):
    ST[k,q]  = KT_h^T QT_h           (K=64 row-packed pairs)
    PT       = exp(SCALE*ST + maskbias[k])   (ScalarE, mask folded into bias)
    OT[hd,q] = V_h^T PT              (M=64 col-packed pairs)
    d[q]     = ones^T PT             (M=1 col-packed)
    OT_norm  = OT * (1/d)            (recip + DMA partition-broadcast + DVE)
  Y_partial[s,H] = OT_norm^T Wo_c   (streamed out per 128-row tile)
"""
import sys

sys.path.insert(0, "/opt/trn_rl_repo")

import numpy as np
import ml_dtypes
from contextlib import ExitStack

B, S, H = 2, 2048, 1024
NH, HD = 16, 64
SCALE = 1.0 / float(np.sqrt(HD))
HPC = 4          # heads per core
HDC = HPC * HD   # 256 per-core head-dim slice
P = 128
KO = H // P      # 8 contraction tiles for the projections
ST_TILES = S // P    # 16
NQ = S // 512        # 4 q-chunks of 512

_BUILT = {}


def _build(dt_name="bfloat16"):
    import concourse.bacc as bacc
    import concourse.mybir as mybir
    import concourse.tile as tile

    DT = getattr(mybir.dt, dt_name)
    F32 = mybir.dt.float32

    nc = bacc.Bacc("TRN2", target_bir_lowering=False, debug=False)

    xt_d = nc.dram_tensor("xt", [H, S], DT, kind="ExternalInput").ap()
    wq_d = nc.dram_tensor("wq", [H, HDC], DT, kind="ExternalInput").ap()
    wk_d = nc.dram_tensor("wk", [H, HDC], DT, kind="ExternalInput").ap()
    wv_d = nc.dram_tensor("wv", [H, HDC], DT, kind="ExternalInput").ap()
    wo_d = nc.dram_tensor("wo", [HDC, H], DT, kind="ExternalInput").ap()
    bqt_d = nc.dram_tensor("bqt", [P, HDC // P], F32, kind="ExternalInput").ap()
    bkt_d = nc.dram_tensor("bkt", [P, HDC // P], F32, kind="ExternalInput").ap()
    bvr_d = nc.dram_tensor("bvr", [P, HDC], F32, kind="ExternalInput").ap()
    mb_d = nc.dram_tensor("mb", [P, ST_TILES], F32, kind="ExternalInput").ap()
    y_d = nc.dram_tensor("y", [S, H], F32, kind="ExternalOutput").ap()

    M2 = HDC // P  # 2 partition-tiles of the per-core head dim

    with tile.TileContext(nc) as tc, ExitStack() as ctx:
        consts = ctx.enter_context(tc.tile_pool(name="consts", bufs=1))
        qkv = ctx.enter_context(tc.tile_pool(name="qkv", bufs=1))
        pt_pool = ctx.enter_context(tc.tile_pool(name="pt", bufs=3))
        sm_pool = ctx.enter_context(tc.tile_pool(name="sm", bufs=3))
        y_pool = ctx.enter_context(tc.tile_pool(name="ysb", bufs=3))
        ps_proj = ctx.enter_context(tc.tile_pool(name="ps_proj", bufs=2, space="PSUM"))
        ps_st = ctx.enter_context(tc.tile_pool(name="ps_st", bufs=2, space="PSUM"))
        ps_ot = ctx.enter_context(tc.tile_pool(name="ps_ot", bufs=1, space="PSUM"))
        ps_d = ctx.enter_context(tc.tile_pool(name="ps_d", bufs=1, space="PSUM"))

        # ---- constants / inputs to SBUF ----
        xt_sb = consts.tile([P, KO, S], DT)
        xt_r = xt_d.rearrange("(ko p) s -> p ko s", p=P)
        for ko in range(KO):
            nc.sync.dma_start(xt_sb[:, ko, :], xt_r[:, ko, :])
        wq_sb = consts.tile([P, KO, HDC], DT)
        nc.sync.dma_start(wq_sb[:], wq_d.rearrange("(ko p) h -> p ko h", p=P))
        wk_sb = consts.tile([P, KO, HDC], DT)
        nc.sync.dma_start(wk_sb[:], wk_d.rearrange("(ko p) h -> p ko h", p=P))
        wv_sb = consts.tile([P, KO, HDC], DT)
        nc.sync.dma_start(wv_sb[:], wv_d.rearrange("(ko p) h -> p ko h", p=P))
        wo_sb = consts.tile([P, M2, H], DT)
        nc.sync.dma_start(wo_sb[:], wo_d.rearrange("(m p) n -> p m n", p=P))
        bqt_sb = consts.tile([P, M2], F32)
        nc.sync.dma_start(bqt_sb[:], bqt_d)
        bkt_sb = consts.tile([P, M2], F32)
        nc.sync.dma_start(bkt_sb[:], bkt_d)
        bvr_sb = consts.tile([P, HDC], F32)
        nc.sync.dma_start(bvr_sb[:], bvr_d)
        mb_sb = consts.tile([P, ST_TILES], F32)
        nc.sync.dma_start(mb_sb[:], mb_d)
        ones_sb = consts.tile([P, 64], DT)
        nc.vector.memset(ones_sb[:], 1.0)

        # ---- Q^T / K^T projections: [hd, s] layout ----
        qt_sb = qkv.tile([P, M2, S], DT)
        kt_sb = qkv.tile([P, M2, S], DT)
        for (w_sb, b_sb, out_sb) in ((wq_sb, bqt_sb, qt_sb), (wk_sb, bkt_sb, kt_sb)):
            for m in range(M2):
                for q in range(NQ):
                    ps = ps_proj.tile([P, 512], F32, tag="ps")
                    for ko in range(KO):
                        nc.tensor.matmul(
                            ps[:],
                            lhsT=w_sb[:, ko, m * P:(m + 1) * P],
                            rhs=xt_sb[:, ko, q * 512:(q + 1) * 512],
                            start=(ko == 0),
                            stop=(ko == KO - 1),
                        )
                    nc.vector.tensor_add(
                        out_sb[:, m, q * 512:(q + 1) * 512],
                        ps[:],
                        b_sb[:, m:m + 1].to_broadcast((P, 512)),
                    )

        # ---- V projection: [s, hd] layout ----
        v_sb = qkv.tile([P, ST_TILES, HDC], DT)
        for st in range(ST_TILES):
            ps_full = ps_proj.tile([P, 512], F32, tag="ps", name="psv")
            ps = ps_full[:, :HDC]
            for ko in range(KO):
                nc.tensor.matmul(
                    ps[:],
                    lhsT=xt_sb[:, ko, st * P:(st + 1) * P],
                    rhs=wv_sb[:, ko, :],
                    start=(ko == 0),
                    stop=(ko == KO - 1),
                )
            nc.vector.tensor_add(v_sb[:, st, :], ps[:], bvr_sb[:])

        # ---- attention + output projection ----
        ot_sb = qkv.tile([P, M2, S], DT)
        for q in range(NQ):
            qs = slice(q * 512, (q + 1) * 512)
            for m in range(M2):
                hA, hB = 2 * m, 2 * m + 1
                ot_ps = ps_ot.tile([P, 512], F32)
                d_ps = ps_d.tile([P, 512], F32)
                for kt in range(ST_TILES):
                    ks = slice(kt * P, (kt + 1) * P)
                    stp = ps_st.tile([P, 1024], F32)
                    # scores^T [k, q] for the head pair (row-packed K=64)
                    nc.tensor.matmul(
                        stp[:, 0:512],
                        lhsT=kt_sb[0:64, m, ks],
                        rhs=qt_sb[0:64, m, qs],
                        start=True, stop=True,
                    )
                    nc.tensor.matmul(
                        stp[:, 512:1024],
                        lhsT=kt_sb[64:128, m, ks],
                        rhs=qt_sb[64:128, m, qs],
                        start=True, stop=True,
                    )
                    pt = pt_pool.tile([P, 1024], DT)
                    nc.scalar.activation(
                        pt[:], stp[:],
                        mybir.ActivationFunctionType.Exp,
                        bias=mb_sb[:, kt:kt + 1],
                        scale=SCALE,
                    )
                    # O^T accumulation, col-packed pair (head A rows 0:64, B 64:128)
                    nc.tensor.matmul(
                        ot_ps[0:64, :],
                        lhsT=v_sb[:, kt, hA * HD:(hA + 1) * HD],
                        rhs=pt[:, 0:512],
                        start=(kt == 0), stop=(kt == ST_TILES - 1),
                    )
                    nc.tensor.matmul(
                        ot_ps[64:128, :],
                        lhsT=v_sb[:, kt, hB * HD:(hB + 1) * HD],
                        rhs=pt[:, 512:1024],
                        start=(kt == 0), stop=(kt == ST_TILES - 1),
                    )
                    # denominators, replicated across 64 rows by 64 ones-columns
                    # (col-packed pair: d_A rows 0:64, d_B rows 64:128)
                    nc.tensor.matmul(
                        d_ps[0:64, :], lhsT=ones_sb[:, 0:64], rhs=pt[:, 0:512],
                        start=(kt == 0), stop=(kt == ST_TILES - 1),
                    )
                    nc.tensor.matmul(
                        d_ps[64:128, :], lhsT=ones_sb[:, 0:64], rhs=pt[:, 512:1024],
                        start=(kt == 0), stop=(kt == ST_TILES - 1),
                    )
                # normalize: ot_sb = ot_ps * (1/d)
                rec = sm_pool.tile([P, 512], F32, tag="rec")
                nc.vector.reciprocal(rec[:], d_ps[:])
                nc.vector.tensor_mul(ot_sb[:, m, qs], ot_ps[:], rec[:])

            # Y partial for the 4 s-tiles covered by this q-chunk
            for st in range(q * 4, q * 4 + 4):
                ss = slice(st * P, (st + 1) * P)
                for n in range(2):
                    ns = slice(n * 512, (n + 1) * 512)
                    yp = ps_proj.tile([P, 512], F32, tag="ps")
                    for m in range(M2):
                        nc.tensor.matmul(
                            yp[:],
                            lhsT=ot_sb[:, m, ss],
                            rhs=wo_sb[:, m, ns],
                            start=(m == 0), stop=(m == M2 - 1),
                        )
                    y_sb = y_pool.tile([P, 512], F32)
                    nc.vector.tensor_copy(y_sb[:], yp[:])
                    nc.sync.dma_start(y_d[ss, ns], y_sb[:])

    nc.compile()
    return nc


def _get_built(dt_name="bfloat16"):
    if dt_name not in _BUILT:
        _BUILT[dt_name] = _build(dt_name)
    return _BUILT[dt_name]


def _prep_core_inputs(c, hidden_states, attention_mask, Wq, bq, Wk, bk, Wv, bv, Wo, bo,
                      np_dt):
    b, g = c // 4, c % 4
    hs = slice(g * HDC, (g + 1) * HDC)
    xt = np.ascontiguousarray(hidden_states[b].T).astype(np_dt)
    mb = np.where(attention_mask[b] == 0, np.float32(-30000.0), np.float32(0.0))
    return {
        "xt": xt,
        "wq": np.ascontiguousarray(Wq[:, hs]).astype(np_dt),
        "wk": np.ascontiguousarray(Wk[:, hs]).astype(np_dt),
        "wv": np.ascontiguousarray(Wv[:, hs]).astype(np_dt),
        "wo": np.ascontiguousarray(Wo[hs, :]).astype(np_dt),
        "bqt": np.ascontiguousarray(bq[hs].reshape(HDC // P, P).T).astype(np.float32),
        "bkt": np.ascontiguousarray(bk[hs].reshape(HDC // P, P).T).astype(np.float32),
        "bvr": np.tile(bv[hs].astype(np.float32), (P, 1)),
        "mb": np.ascontiguousarray(mb.astype(np.float32).reshape(ST_TILES, P).T),
    }


def kernel(hidden_states, attention_mask, Wq, bq, Wk, bk, Wv, bv, Wo, bo,
           _trace=False, _trace_kwargs=None):
    from concourse.bass_utils import run_bass_kernel_spmd

    hidden_states = np.asarray(hidden_states, np.float32)
    attention_mask = np.asarray(attention_mask)
    Wq, bq = np.asarray(Wq, np.float32), np.asarray(bq, np.float32)
    Wk, bk = np.asarray(Wk, np.float32), np.asarray(bk, np.float32)
    Wv, bv = np.asarray(Wv, np.float32), np.asarray(bv, np.float32)
    Wo, bo = np.asarray(Wo, np.float32), np.asarray(bo, np.float32)

    nc = _get_built()
    np_dt = ml_dtypes.bfloat16
    in_maps = [
        _prep_core_inputs(c, hidden_states, attention_mask,
                          Wq, bq, Wk, bk, Wv, bv, Wo, bo, np_dt)
        for c in range(8)
    ]
    kwargs = {}
    if _trace:
        kwargs["trace"] = True
        if _trace_kwargs:
            kwargs.update(_trace_kwargs)
    res = run_bass_kernel_spmd(nc, in_maps, core_ids=list(range(8)), **kwargs)
    out = np.empty((B, S, H), np.float32)
    for b in range(B):
        acc = res.results[4 * b]["y"].astype(np.float32).copy()
        for c in range(4 * b + 1, 4 * b + 4):
            acc += res.results[c]["y"]
        out[b] = acc + bo[None, :]
    if _trace:
        return out, res
    return out


# revision 8
# speedup vs baseline: 1.1817x; 1.1817x over previous
"""Multi-head attention (B=2, S=2048, H=1024, 16 heads x 64) on 8 trn2 cores.

Sharding: core c handles batch b=c//4 and the 4 heads [4*(c%4) .. 4*(c%4)+3]
(tensor-parallel over the hd=256 column slice of Wq/Wk/Wv and the matching
row slice of Wo).  Each core computes a rank-256 partial of the output
projection for its batch; the host sums the 4 partials per batch and adds bo.

Device kernel (per core, all in bf16 matmuls with fp32 PSUM accumulate):
  QT[hd,s] = Wq_c^T X_b^T   (lhsT=Wq nat. layout, rhs=X^T prepped on host)
  KT[hd,s], V[s,hd] similarly.
  Per head pair (2 heads packed in the 128-partition dim):
    ST[k,q]  = KT_h^T QT_h           (K=64 row-packed pairs)
    PT       = exp(SCALE*ST + maskbias[k])   (ScalarE, mask folded into bias)
    OT[hd,q] = V_h^T PT              (M=64 col-packed pairs)
    d[q]     = ones^T PT             (M=1 col-packed)
    OT_norm  = OT * (1/d)            (recip + DMA partition-broadcast + DVE)
  Y_partial[s,H] = OT_norm^T Wo_c   (streamed out per 128-row tile)
"""
import sys

sys.path.insert(0, "/opt/trn_rl_repo")

import numpy as np
import ml_dtypes
from contextlib import ExitStack

B, S, H = 2, 2048, 1024
NH, HD = 16, 64
SCALE = 1.0 / float(np.sqrt(HD))
HPC = 4          # heads per core
HDC = HPC * HD   # 256 per-core head-dim slice
P = 128
KO = H // P      # 8 contraction tiles for the projections
ST_TILES = S // P    # 16
NQ = S // 512        # 4 q-chunks of 512

_BUILT = {}


def _build(dt_name="bfloat16"):
    import concourse.bacc as bacc
    import concourse.mybir as mybir
    import concourse.tile as tile

    DT = getattr(mybir.dt, dt_name)
    F32 = mybir.dt.float32

    nc = bacc.Bacc("TRN2", target_bir_lowering=False, debug=False)

    xt_d = nc.dram_tensor("xt", [H, S], DT, kind="ExternalInput").ap()
    wq_d = nc.dram_tensor("wq", [H, HDC], DT, kind="ExternalInput").ap()
    wk_d = nc.dram_tensor("wk", [H, HDC], DT, kind="ExternalInput").ap()
    wv_d = nc.dram_tensor("wv", [H, HDC], DT, kind="ExternalInput").ap()
    wo_d = nc.dram_tensor("wo", [HDC, H], DT, kind="ExternalInput").ap()
    bqt_d = nc.dram_tensor("bqt", [P, HDC // P], F32, kind="ExternalInput").ap()
    bkt_d = nc.dram_tensor("bkt", [P, HDC // P], F32, kind="ExternalInput").ap()
    bvr_d = nc.dram_tensor("bvr", [P, HDC], F32, kind="ExternalInput").ap()
    mb_d = nc.dram_tensor("mb", [P, ST_TILES], F32, kind="ExternalInput").ap()
    y_d = nc.dram_tensor("y", [S, H], F32, kind="ExternalOutput").ap()

    M2 = HDC // P  # 2 partition-tiles of the per-core head dim

    with tile.TileContext(nc) as tc, ExitStack() as ctx:
        consts = ctx.enter_context(tc.tile_pool(name="consts", bufs=1))
        qkv = ctx.enter_context(tc.tile_pool(name="qkv", bufs=1))
        pt_pool = ctx.enter_context(tc.tile_pool(name="pt", bufs=3))
        sm_pool = ctx.enter_context(tc.tile_pool(name="sm", bufs=3))
        y_pool = ctx.enter_context(tc.tile_pool(name="ysb", bufs=3))
        ps_proj = ctx.enter_context(tc.tile_pool(name="ps_proj", bufs=2, space="PSUM"))
        ps_st = ctx.enter_context(tc.tile_pool(name="ps_st", bufs=2, space="PSUM"))
        ps_ot = ctx.enter_context(tc.tile_pool(name="ps_ot", bufs=1, space="PSUM"))
        ps_d = ctx.enter_context(tc.tile_pool(name="ps_d", bufs=1, space="PSUM"))

        # ---- constants / inputs to SBUF ----
        xt_sb = consts.tile([P, KO, S], DT)
        xt_r = xt_d.rearrange("(ko p) s -> p ko s", p=P)
        for ko in range(KO):
            nc.sync.dma_start(xt_sb[:, ko, :], xt_r[:, ko, :])
        wq_sb = consts.tile([P, KO, HDC], DT)
        nc.sync.dma_start(wq_sb[:], wq_d.rearrange("(ko p) h -> p ko h", p=P))
        wk_sb = consts.tile([P, KO, HDC], DT)
        nc.sync.dma_start(wk_sb[:], wk_d.rearrange("(ko p) h -> p ko h", p=P))
        wv_sb = consts.tile([P, KO, HDC], DT)
        nc.sync.dma_start(wv_sb[:], wv_d.rearrange("(ko p) h -> p ko h", p=P))
        wo_sb = consts.tile([P, M2, H], DT)
        nc.sync.dma_start(wo_sb[:], wo_d.rearrange("(m p) n -> p m n", p=P))
        bqt_sb = consts.tile([P, M2], F32)
        nc.sync.dma_start(bqt_sb[:], bqt_d)
        bkt_sb = consts.tile([P, M2], F32)
        nc.sync.dma_start(bkt_sb[:], bkt_d)
        bvr_sb = consts.tile([P, HDC], F32)
        nc.sync.dma_start(bvr_sb[:], bvr_d)
        mb_sb = consts.tile([P, ST_TILES], F32)
        nc.sync.dma_start(mb_sb[:], mb_d)
        ones_sb = consts.tile([P, 64], DT)
        nc.vector.memset(ones_sb[:], 1.0)

        # ---- Q^T / K^T projections: [hd, s] layout ----
        qt_sb = qkv.tile([P, M2, S], DT)
        kt_sb = qkv.tile([P, M2, S], DT)
        for (w_sb, b_sb, out_sb) in ((wq_sb, bqt_sb, qt_sb), (wk_sb, bkt_sb, kt_sb)):
            for m in range(M2):
                for q in range(NQ):
                    ps = ps_proj.tile([P, 512], F32, tag="ps")
                    for ko in range(KO):
                        nc.tensor.matmul(
                            ps[:],
                            lhsT=w_sb[:, ko, m * P:(m + 1) * P],
                            rhs=xt_sb[:, ko, q * 512:(q + 1) * 512],
                            start=(ko == 0),
                            stop=(ko == KO - 1),
                        )
                    nc.vector.tensor_add(
                        out_sb[:, m, q * 512:(q + 1) * 512],
                        ps[:],
                        b_sb[:, m:m + 1].to_broadcast((P, 512)),
                    )

        # ---- V projection: [s, hd] layout ----
        v_sb = qkv.tile([P, ST_TILES, HDC], DT)
        for st in range(ST_TILES):
            ps_full = ps_proj.tile([P, 512], F32, tag="ps", name="psv")
            ps = ps_full[:, :HDC]
            for ko in range(KO):
                nc.tensor.matmul(
                    ps[:],
                    lhsT=xt_sb[:, ko, st * P:(st + 1) * P],
                    rhs=wv_sb[:, ko, :],
                    start=(ko == 0),
                    stop=(ko == KO - 1),
                )
            nc.vector.tensor_add(v_sb[:, st, :], ps[:], bvr_sb[:])

        # ---- attention + output projection (software-pipelined) ----
        # Emission order per iteration: exp(i) [ACT], ST(i+1) [PE], PV/d(i)
        # [PE] — so the next scores matmuls sit AHEAD of the exp-dependent PV
        # in the PE FIFO and the engine never head-of-line blocks on ACT.
        ot_sb = qkv.tile([P, M2, S], DT)
        seq = [(q, m, kt) for q in range(NQ) for m in range(M2)
               for kt in range(ST_TILES)]

        def st_mms(q, m, kt):
            ks = slice(kt * P, (kt + 1) * P)
            qs = slice(q * 512, (q + 1) * 512)
            stp = ps_st.tile([P, 1024], F32, name="stp", tag="stp")
            nc.tensor.matmul(
                stp[:, 0:512],
                lhsT=kt_sb[0:64, m, ks], rhs=qt_sb[0:64, m, qs],
                start=True, stop=True,
            )
            nc.tensor.matmul(
                stp[:, 512:1024],
                lhsT=kt_sb[64:128, m, ks], rhs=qt_sb[64:128, m, qs],
                start=True, stop=True,
            )
            return stp

        y_backlog = []

        def emit_y(q):
            for st in range(q * 4, q * 4 + 4):
                for n in range(2):
                    y_backlog.append((st, n))

        def drain_y(count):
            for _ in range(count):
                if not y_backlog:
                    return
                st, n = y_backlog.pop(0)
                ss = slice(st * P, (st + 1) * P)
                ns = slice(n * 512, (n + 1) * 512)
                yp = ps_proj.tile([P, 512], F32, tag="ps", name="yp")
                for m in range(M2):
                    nc.tensor.matmul(
                        yp[:],
                        lhsT=ot_sb[:, m, ss], rhs=wo_sb[:, m, ns],
                        start=(m == 0), stop=(m == M2 - 1),
                    )
                y_sb = y_pool.tile([P, 512], F32, name="y_sb")
                nc.vector.tensor_copy(y_sb[:], yp[:])
                nc.sync.dma_start(y_d[ss, ns], y_sb[:])

        ot_ps = d_ps = None
        stp_cur = st_mms(*seq[0])
        for i, (q, m, kt) in enumerate(seq):
            qs = slice(q * 512, (q + 1) * 512)
            hA, hB = 2 * m, 2 * m + 1
            if kt == 0:
                ot_ps = ps_ot.tile([P, 512], F32, name="ot_ps")
                d_ps = ps_d.tile([P, 512], F32, name="d_ps")
            pt = pt_pool.tile([P, 1024], DT, name="pt")
            nc.scalar.activation(
                pt[:], stp_cur[:],
                mybir.ActivationFunctionType.Exp,
                bias=mb_sb[:, kt:kt + 1],
                scale=SCALE,
            )
            if i + 1 < len(seq):
                stp_next = st_mms(*seq[i + 1])
            # O^T accumulation, col-packed pair (head A rows 0:64, B 64:128)
            nc.tensor.matmul(
                ot_ps[0:64, :],
                lhsT=v_sb[:, kt, hA * HD:(hA + 1) * HD], rhs=pt[:, 0:512],
                start=(kt == 0), stop=(kt == ST_TILES - 1),
            )
            nc.tensor.matmul(
                ot_ps[64:128, :],
                lhsT=v_sb[:, kt, hB * HD:(hB + 1) * HD], rhs=pt[:, 512:1024],
                start=(kt == 0), stop=(kt == ST_TILES - 1),
            )
            # denominators, replicated across 64 rows by 64 ones-columns
            nc.tensor.matmul(
                d_ps[0:64, :], lhsT=ones_sb[:, 0:64], rhs=pt[:, 0:512],
                start=(kt == 0), stop=(kt == ST_TILES - 1),
            )
            nc.tensor.matmul(
                d_ps[64:128, :], lhsT=ones_sb[:, 0:64], rhs=pt[:, 512:1024],
                start=(kt == 0), stop=(kt == ST_TILES - 1),
            )
            if kt == ST_TILES - 1:
                # normalize: ot_sb = ot_ps * (1/d)
                rec = sm_pool.tile([P, 512], F32, tag="rec", name="rec")
                nc.vector.reciprocal_approx_fast(rec[:], d_ps[:])
                nc.vector.tensor_mul(ot_sb[:, m, qs], ot_ps[:], rec[:])
                if m == M2 - 1:
                    emit_y(q)
            drain_y(1)
            stp_cur = stp_next
        drain_y(len(y_backlog))

    nc.compile()
    return nc


def _get_built(dt_name="bfloat16"):
    if dt_name not in _BUILT:
        _BUILT[dt_name] = _build(dt_name)
    return _BUILT[dt_name]


def _prep_core_inputs(c, hidden_states, attention_mask, Wq, bq, Wk, bk, Wv, bv, Wo, bo,
                      np_dt):
    b, g = c // 4, c % 4
    hs = slice(g * HDC, (g + 1) * HDC)
    xt = np.ascontiguousarray(hidden_states[b].T).astype(np_dt)
    mb = np.where(attention_mask[b] == 0, np.float32(-30000.0), np.float32(0.0))
    return {
        "xt": xt,
        "wq": np.ascontiguousarray(Wq[:, hs]).astype(np_dt),
        "wk": np.ascontiguousarray(Wk[:, hs]).astype(np_dt),
        "wv": np.ascontiguousarray(Wv[:, hs]).astype(np_dt),
        "wo": np.ascontiguousarray(Wo[hs, :]).astype(np_dt),
        "bqt": np.ascontiguousarray(bq[hs].reshape(HDC // P, P).T).astype(np.float32),
        "bkt": np.ascontiguousarray(bk[hs].reshape(HDC // P, P).T).astype(np.float32),
        "bvr": np.tile(bv[hs].astype(np.float32), (P, 1)),
        "mb": np.ascontiguousarray(mb.astype(np.float32).reshape(ST_TILES, P).T),
    }


def kernel(hidden_states, attention_mask, Wq, bq, Wk, bk, Wv, bv, Wo, bo,
           _trace=False, _trace_kwargs=None):
    from concourse.bass_utils import run_bass_kernel_spmd

    hidden_states = np.asarray(hidden_states, np.float32)
    attention_mask = np.asarray(attention_mask)
    Wq, bq = np.asarray(Wq, np.float32), np.asarray(bq, np.float32)
    Wk, bk = np.asarray(Wk, np.float32), np.asarray(bk, np.float32)
    Wv, bv = np.asarray(Wv, np.float32), np.asarray(bv, np.float32)
    Wo, bo = np.asarray(Wo, np.float32), np.asarray(bo, np.float32)

    nc = _get_built()
    np_dt = ml_dtypes.bfloat16
    in_maps = [
        _prep_core_inputs(c, hidden_states, attention_mask,
                          Wq, bq, Wk, bk, Wv, bv, Wo, bo, np_dt)
        for c in range(8)
    ]
    kwargs = {}
    if _trace:
        kwargs["trace"] = True
        if _trace_kwargs:
            kwargs.update(_trace_kwargs)
    res = run_bass_kernel_spmd(nc, in_maps, core_ids=list(range(8)), **kwargs)
    out = np.empty((B, S, H), np.float32)
    for b in range(B):
        acc = res.results[4 * b]["y"].astype(np.float32).copy()
        for c in range(4 * b + 1, 4 * b + 4):
            acc += res.results[c]["y"]
        out[b] = acc + bo[None, :]
    if _trace:
        return out, res
    return out


# revision 10
# speedup vs baseline: 1.4408x; 1.2193x over previous
"""Multi-head attention (B=2, S=2048, H=1024, 16 heads x 64) on 8 trn2 cores.

Sharding: core c handles batch b=c//4 and the 4 heads [4*(c%4) .. 4*(c%4)+3]
(tensor-parallel over the hd=256 column slice of Wq/Wk/Wv and the matching
row slice of Wo).  Each core computes a rank-256 partial of the output
projection for its batch; the host sums the 4 partials per batch and adds bo.

Device kernel (per core, bf16 matmuls with fp32 PSUM accumulate):
  QT[hd,s] = Wq_c^T X_b^T   (lhsT=Wq nat. layout, rhs=X^T prepped on host)
  KT[hd,s], V[s,hd] similarly.
  Per head pair (2 heads packed in the 128-partition dim):
    ST[k,q]  = KT_h^T QT_h           (K=64 row-packed pairs)
    PT       = exp(SCALE*ST + maskbias[k])   (ScalarE, mask folded into bias)
    OT[hd,q] = V_h^T PT              (M=64 col-packed pairs)
    d[q]     = ones64^T PT           (64 ones-columns -> d replicated per row)
    OT_norm  = OT * (1/d)            (reciprocal_approx_fast + DVE mult)
  Y_partial[s,H] = OT_norm^T Wo_c   (streamed out per 128-row tile)

Scheduling: one flat software pipeline paced by ScalarE's exp.  Projection
groups (QT/KT/V) and output-projection tiles are emitted as backlog items
drained between attention iterations, so the PE FIFO never head-of-line
blocks and startup/tail overlap the steady state.
"""
import sys

sys.path.insert(0, "/opt/trn_rl_repo")

import numpy as np
import ml_dtypes
from contextlib import ExitStack

B, S, H = 2, 2048, 1024
NH, HD = 16, 64
SCALE = 1.0 / float(np.sqrt(HD))
HPC = 4          # heads per core
HDC = HPC * HD   # 256 per-core head-dim slice
P = 128
KO = H // P      # 8 contraction tiles for the projections
ST_TILES = S // P    # 16
NQ = S // 512        # 4 q-chunks of 512
M2 = HDC // P        # 2 partition-tiles of the per-core head dim

_BUILT = {}


def _build(dt_name="bfloat16"):
    import concourse.bacc as bacc
    import concourse.mybir as mybir
    import concourse.tile as tile

    DT = getattr(mybir.dt, dt_name)
    F32 = mybir.dt.float32

    nc = bacc.Bacc("TRN2", target_bir_lowering=False, debug=False)

    xt_d = nc.dram_tensor("xt", [H, S], DT, kind="ExternalInput").ap()
    wq_d = nc.dram_tensor("wq", [H, HDC], DT, kind="ExternalInput").ap()
    wk_d = nc.dram_tensor("wk", [H, HDC], DT, kind="ExternalInput").ap()
    wv_d = nc.dram_tensor("wv", [H, HDC], DT, kind="ExternalInput").ap()
    wo_d = nc.dram_tensor("wo", [HDC, H], DT, kind="ExternalInput").ap()
    bqt_d = nc.dram_tensor("bqt", [P, M2], F32, kind="ExternalInput").ap()
    bkt_d = nc.dram_tensor("bkt", [P, M2], F32, kind="ExternalInput").ap()
    bvr_d = nc.dram_tensor("bvr", [P, HDC], F32, kind="ExternalInput").ap()
    mb_d = nc.dram_tensor("mb", [P, ST_TILES], F32, kind="ExternalInput").ap()
    y_d = nc.dram_tensor("y", [S, H], F32, kind="ExternalOutput").ap()

    with tile.TileContext(nc) as tc, ExitStack() as ctx:
        consts = ctx.enter_context(tc.tile_pool(name="consts", bufs=1))
        qkv = ctx.enter_context(tc.tile_pool(name="qkv", bufs=1))
        pt_pool = ctx.enter_context(tc.tile_pool(name="pt", bufs=3))
        sm_pool = ctx.enter_context(tc.tile_pool(name="sm", bufs=3))
        y_pool = ctx.enter_context(tc.tile_pool(name="ysb", bufs=3))
        ps_proj = ctx.enter_context(tc.tile_pool(name="ps_proj", bufs=2, space="PSUM"))
        ps_st = ctx.enter_context(tc.tile_pool(name="ps_st", bufs=2, space="PSUM"))
        ps_ot = ctx.enter_context(tc.tile_pool(name="ps_ot", bufs=1, space="PSUM"))
        ps_d = ctx.enter_context(tc.tile_pool(name="ps_d", bufs=1, space="PSUM"))

        # ---- input DMAs (small consts first, xt by s-chunk, wo last) ----
        bqt_sb = consts.tile([P, M2], F32)
        nc.sync.dma_start(bqt_sb[:], bqt_d)
        bkt_sb = consts.tile([P, M2], F32)
        nc.sync.dma_start(bkt_sb[:], bkt_d)
        bvr_sb = consts.tile([P, HDC], F32)
        nc.sync.dma_start(bvr_sb[:], bvr_d)
        mb_sb = consts.tile([P, ST_TILES], F32)
        nc.sync.dma_start(mb_sb[:], mb_d)
        ones_sb = consts.tile([P, 64], DT)
        nc.vector.memset(ones_sb[:], 1.0)

        wk_sb = consts.tile([P, KO, HDC], DT)
        nc.sync.dma_start(wk_sb[:], wk_d.rearrange("(ko p) h -> p ko h", p=P))
        wq_sb = consts.tile([P, KO, HDC], DT)
        nc.sync.dma_start(wq_sb[:], wq_d.rearrange("(ko p) h -> p ko h", p=P))
        wv_sb = consts.tile([P, KO, HDC], DT)
        nc.sync.dma_start(wv_sb[:], wv_d.rearrange("(ko p) h -> p ko h", p=P))

        xt_sb = consts.tile([P, KO, S], DT)
        xt_r = xt_d.rearrange("(ko p) s -> p ko s", p=P)
        for c in range(NQ):
            cs = slice(c * 512, (c + 1) * 512)
            for ko in range(KO):
                nc.sync.dma_start(xt_sb[:, ko, cs], xt_r[:, ko, cs])

        wo_sb = consts.tile([P, M2, H], DT)
        nc.sync.dma_start(wo_sb[:], wo_d.rearrange("(m p) n -> p m n", p=P))

        qt_sb = qkv.tile([P, M2, S], DT)
        kt_sb = qkv.tile([P, M2, S], DT)
        v_sb = qkv.tile([P, ST_TILES, HDC], DT)
        ot_sb = qkv.tile([P, M2, S], DT)

        # ---- projection group emitters ----
        def proj_qk(w_sb, b_sb, out_sb, m, q):
            qs = slice(q * 512, (q + 1) * 512)
            ps = ps_proj.tile([P, 512], F32, tag="ps", name="ps_qk")
            for ko in range(KO):
                nc.tensor.matmul(
                    ps[:],
                    lhsT=w_sb[:, ko, m * P:(m + 1) * P],
                    rhs=xt_sb[:, ko, qs],
                    start=(ko == 0), stop=(ko == KO - 1),
                )
            nc.vector.tensor_add(
                out_sb[:, m, qs], ps[:],
                b_sb[:, m:m + 1].to_broadcast((P, 512)),
            )

        def proj_v(st):
            ps_full = ps_proj.tile([P, 512], F32, tag="ps", name="ps_v")
            ps = ps_full[:, :HDC]
            for ko in range(KO):
                nc.tensor.matmul(
                    ps[:],
                    lhsT=xt_sb[:, ko, st * P:(st + 1) * P],
                    rhs=wv_sb[:, ko, :],
                    start=(ko == 0), stop=(ko == KO - 1),
                )
            nc.vector.tensor_add(v_sb[:, st, :], ps[:], bvr_sb[:])

        def emit_y_tile(st, n):
            ss = slice(st * P, (st + 1) * P)
            ns = slice(n * 512, (n + 1) * 512)
            yp = ps_proj.tile([P, 512], F32, tag="ps", name="yp")
            for m in range(M2):
                nc.tensor.matmul(
                    yp[:],
                    lhsT=ot_sb[:, m, ss], rhs=wo_sb[:, m, ns],
                    start=(m == 0), stop=(m == M2 - 1),
                )
            y_sb = y_pool.tile([P, 512], F32, name="y_sb")
            nc.vector.tensor_copy(y_sb[:], yp[:])
            nc.sync.dma_start(y_d[ss, ns], y_sb[:])

        # ---- backlog of work drained through the pipeline ----
        # items: (deadline_iter, kind, args); kept sorted by deadline.
        backlog = []

        def add(deadline, kind, *args):
            backlog.append((deadline, kind, args))

        def run_item(kind, args):
            if kind == "qk":
                w_sb, b_sb, out_sb, m, q = args
                proj_qk(w_sb, b_sb, out_sb, m, q)
            elif kind == "v":
                proj_v(args[0])
            elif kind == "y":
                emit_y_tile(*args)

        def drain(i, budget):
            """Emit backlog items: all whose deadline is within 2 iters, then
            up to `budget` more."""
            backlog.sort(key=lambda t: t[0])
            n = 0
            while backlog:
                dl, kind, args = backlog[0]
                if dl <= i + 2 or n < budget:
                    backlog.pop(0)
                    run_item(kind, args)
                    n += 1
                else:
                    break

        # attention group order: m-outer so m=1 projections have slack
        seq = [(q, m, kt) for m in range(M2) for q in range(NQ)
               for kt in range(ST_TILES)]
        giter = {}  # (q, m) -> start iter
        for i, (q, m, kt) in enumerate(seq):
            if kt == 0:
                giter[(q, m)] = i

        # prefix: just enough for the pipeline to start
        proj_qk(wk_sb, bkt_sb, kt_sb, 0, 0)   # KT m0 chunk 0 (kt 0-3)
        proj_qk(wq_sb, bqt_sb, qt_sb, 0, 0)   # QT m0 q0
        proj_v(0)
        proj_v(1)

        # backlog deadlines
        for m in range(M2):
            for j in range(NQ):
                if (m, j) != (0, 0):
                    # KT chunk j needed by kt=4j of every group of this m
                    add(giter[(0, m)] + 4 * j if (m, j) != (0, 0) else 0,
                        "qk", wk_sb, bkt_sb, kt_sb, m, j)
                if (m, j) != (0, 0):
                    add(giter[(j, m)], "qk", wq_sb, bqt_sb, qt_sb, m, j)
        for st in range(2, ST_TILES):
            add(st, "v", st)

        # ---- flat attention pipeline ----
        def st_mms(q, m, kt):
            ks = slice(kt * P, (kt + 1) * P)
            qs = slice(q * 512, (q + 1) * 512)
            stp = ps_st.tile([P, 1024], F32, name="stp", tag="stp")
            nc.tensor.matmul(
                stp[:, 0:512],
                lhsT=kt_sb[0:64, m, ks], rhs=qt_sb[0:64, m, qs],
                start=True, stop=True,
            )
            nc.tensor.matmul(
                stp[:, 512:1024],
                lhsT=kt_sb[64:128, m, ks], rhs=qt_sb[64:128, m, qs],
                start=True, stop=True,
            )
            return stp

        ot_ps = d_ps = None
        stp_cur = st_mms(*seq[0])
        for i, (q, m, kt) in enumerate(seq):
            qs = slice(q * 512, (q + 1) * 512)
            hA, hB = 2 * m, 2 * m + 1
            if kt == 0:
                ot_ps = ps_ot.tile([P, 512], F32, name="ot_ps")
                d_ps = ps_d.tile([P, 512], F32, name="d_ps")
            pt = pt_pool.tile([P, 1024], DT, name="pt")
            nc.scalar.activation(
                pt[:], stp_cur[:],
                mybir.ActivationFunctionType.Exp,
                bias=mb_sb[:, kt:kt + 1],
                scale=SCALE,
            )
            if i + 1 < len(seq):
                stp_next = st_mms(*seq[i + 1])
            # O^T accumulation, col-packed pair (head A rows 0:64, B 64:128)
            nc.tensor.matmul(
                ot_ps[0:64, :],
                lhsT=v_sb[:, kt, hA * HD:(hA + 1) * HD], rhs=pt[:, 0:512],
                start=(kt == 0), stop=(kt == ST_TILES - 1),
            )
            nc.tensor.matmul(
                ot_ps[64:128, :],
                lhsT=v_sb[:, kt, hB * HD:(hB + 1) * HD], rhs=pt[:, 512:1024],
                start=(kt == 0), stop=(kt == ST_TILES - 1),
            )
            # denominators, replicated across 64 rows by 64 ones-columns
            nc.tensor.matmul(
                d_ps[0:64, :], lhsT=ones_sb[:, 0:64], rhs=pt[:, 0:512],
                start=(kt == 0), stop=(kt == ST_TILES - 1),
            )
            nc.tensor.matmul(
                d_ps[64:128, :], lhsT=ones_sb[:, 0:64], rhs=pt[:, 512:1024],
                start=(kt == 0), stop=(kt == ST_TILES - 1),
            )
            if kt == ST_TILES - 1:
                rec = sm_pool.tile([P, 512], F32, tag="rec", name="rec")
                nc.vector.reciprocal_approx_fast(rec[:], d_ps[:])
                nc.vector.tensor_mul(ot_sb[:, m, qs], ot_ps[:], rec[:])
                if m == M2 - 1:
                    idx = 0
                    for st in range(q * 4, q * 4 + 4):
                        for n in range(2):
                            add(i + 2 + 2 * idx, "y", st, n)
                            idx += 1
            drain(i, 1)
            stp_cur = stp_next
        drain(10 ** 9, 10 ** 9)

    nc.compile()
    return nc


def _get_built(dt_name="bfloat16"):
    if dt_name not in _BUILT:
        _BUILT[dt_name] = _build(dt_name)
    return _BUILT[dt_name]


def _prep_core_inputs(c, hidden_states, attention_mask, Wq, bq, Wk, bk, Wv, bv, Wo, bo,
                      np_dt):
    b, g = c // 4, c % 4
    hs = slice(g * HDC, (g + 1) * HDC)
    xt = np.ascontiguousarray(hidden_states[b].T).astype(np_dt)
    mb = np.where(attention_mask[b] == 0, np.float32(-30000.0), np.float32(0.0))
    return {
        "xt": xt,
        "wq": np.ascontiguousarray(Wq[:, hs]).astype(np_dt),
        "wk": np.ascontiguousarray(Wk[:, hs]).astype(np_dt),
        "wv": np.ascontiguousarray(Wv[:, hs]).astype(np_dt),
        "wo": np.ascontiguousarray(Wo[hs, :]).astype(np_dt),
        "bqt": np.ascontiguousarray(bq[hs].reshape(M2, P).T).astype(np.float32),
        "bkt": np.ascontiguousarray(bk[hs].reshape(M2, P).T).astype(np.float32),
        "bvr": np.tile(bv[hs].astype(np.float32), (P, 1)),
        "mb": np.ascontiguousarray(mb.astype(np.float32).reshape(ST_TILES, P).T),
    }


def kernel(hidden_states, attention_mask, Wq, bq, Wk, bk, Wv, bv, Wo, bo,
           _trace=False, _trace_kwargs=None):
    from concourse.bass_utils import run_bass_kernel_spmd

    hidden_states = np.asarray(hidden_states, np.float32)
    attention_mask = np.asarray(attention_mask)
    Wq, bq = np.asarray(Wq, np.float32), np.asarray(bq, np.float32)
    Wk, bk = np.asarray(Wk, np.float32), np.asarray(bk, np.float32)
    Wv, bv = np.asarray(Wv, np.float32), np.asarray(bv, np.float32)
    Wo, bo = np.asarray(Wo, np.float32), np.asarray(bo, np.float32)

    nc = _get_built()
    np_dt = ml_dtypes.bfloat16
    in_maps = [
        _prep_core_inputs(c, hidden_states, attention_mask,
                          Wq, bq, Wk, bk, Wv, bv, Wo, bo, np_dt)
        for c in range(8)
    ]
    kwargs = {}
    if _trace:
        kwargs["trace"] = True
        if _trace_kwargs:
            kwargs.update(_trace_kwargs)
    res = run_bass_kernel_spmd(nc, in_maps, core_ids=list(range(8)), **kwargs)
    out = np.empty((B, S, H), np.float32)
    for b in range(B):
        acc = res.results[4 * b]["y"].astype(np.float32).copy()
        for c in range(4 * b + 1, 4 * b + 4):
            acc += res.results[c]["y"]
        out[b] = acc + bo[None, :]
    if _trace:
        return out, res
    return out


# revision 11
# speedup vs baseline: 1.4680x; 1.0188x over previous
"""Multi-head attention (B=2, S=2048, H=1024, 16 heads x 64) on 8 trn2 cores.

Sharding: core c handles batch b=c//4 and the 4 heads [4*(c%4) .. 4*(c%4)+3]
(tensor-parallel over the hd=256 column slice of Wq/Wk/Wv and the matching
row slice of Wo).  Each core computes a rank-256 partial of the output
projection for its batch; the host sums the 4 partials per batch and adds bo.

Device kernel (per core, bf16 matmuls with fp32 PSUM accumulate):
  QT[hd,s] = Wq_c^T X_b^T   (lhsT=Wq nat. layout, rhs=X^T prepped on host)
  KT[hd,s], V[s,hd] similarly.
  Per head pair (2 heads packed in the 128-partition dim):
    ST[k,q]  = KT_h^T QT_h           (K=64 row-packed pairs)
    PT       = exp(SCALE*ST + maskbias[k])   (ScalarE, mask folded into bias)
    OT[hd,q] = V_h^T PT              (M=64 col-packed pairs)
    d[q]     = ones64^T PT           (64 ones-columns -> d replicated per row)
    OT_norm  = OT * (1/d)            (reciprocal_approx_fast + DVE mult)
  Y_partial[s,H] = OT_norm^T Wo_c   (streamed out per 128-row tile)

Scheduling: one flat software pipeline paced by ScalarE's exp.  Projection
groups (QT/KT/V) and output-projection tiles are emitted as backlog items
drained between attention iterations, so the PE FIFO never head-of-line
blocks and startup/tail overlap the steady state.
"""
import sys

sys.path.insert(0, "/opt/trn_rl_repo")

import numpy as np
import ml_dtypes
from contextlib import ExitStack

B, S, H = 2, 2048, 1024
NH, HD = 16, 64
SCALE = 1.0 / float(np.sqrt(HD))
HPC = 4          # heads per core
HDC = HPC * HD   # 256 per-core head-dim slice
P = 128
KO = H // P      # 8 contraction tiles for the projections
ST_TILES = S // P    # 16
NQ = S // 512        # 4 q-chunks of 512
M2 = HDC // P        # 2 partition-tiles of the per-core head dim

_BUILT = {}


def _build(dt_name="bfloat16"):
    import concourse.bacc as bacc
    import concourse.mybir as mybir
    import concourse.tile as tile

    DT = getattr(mybir.dt, dt_name)
    F32 = mybir.dt.float32

    nc = bacc.Bacc("TRN2", target_bir_lowering=False, debug=False)

    xt_d = nc.dram_tensor("xt", [H, S], DT, kind="ExternalInput").ap()
    wq_d = nc.dram_tensor("wq", [H, HDC], DT, kind="ExternalInput").ap()
    wk_d = nc.dram_tensor("wk", [H, HDC], DT, kind="ExternalInput").ap()
    wv_d = nc.dram_tensor("wv", [H, HDC], DT, kind="ExternalInput").ap()
    wo_d = nc.dram_tensor("wo", [HDC, H], DT, kind="ExternalInput").ap()
    bqt_d = nc.dram_tensor("bqt", [P, M2], F32, kind="ExternalInput").ap()
    bkt_d = nc.dram_tensor("bkt", [P, M2], F32, kind="ExternalInput").ap()
    bvr_d = nc.dram_tensor("bvr", [P, HDC], F32, kind="ExternalInput").ap()
    mb_d = nc.dram_tensor("mb", [P, ST_TILES], F32, kind="ExternalInput").ap()
    y_d = nc.dram_tensor("y", [S, H], F32, kind="ExternalOutput").ap()

    with tile.TileContext(nc) as tc, ExitStack() as ctx:
        consts = ctx.enter_context(tc.tile_pool(name="consts", bufs=1))
        qkv = ctx.enter_context(tc.tile_pool(name="qkv", bufs=1))
        pt_pool = ctx.enter_context(tc.tile_pool(name="pt", bufs=5))
        sm_pool = ctx.enter_context(tc.tile_pool(name="sm", bufs=4))
        y_pool = ctx.enter_context(tc.tile_pool(name="ysb", bufs=4))
        ps_proj = ctx.enter_context(tc.tile_pool(name="ps_proj", bufs=2, space="PSUM"))
        ps_st = ctx.enter_context(tc.tile_pool(name="ps_st", bufs=2, space="PSUM"))
        ps_ot = ctx.enter_context(tc.tile_pool(name="ps_ot", bufs=1, space="PSUM"))
        ps_d = ctx.enter_context(tc.tile_pool(name="ps_d", bufs=1, space="PSUM"))

        # ---- input DMAs (small consts first, xt by s-chunk, wo last) ----
        bqt_sb = consts.tile([P, M2], F32)
        nc.gpsimd.dma_start(bqt_sb[:], bqt_d)
        bkt_sb = consts.tile([P, M2], F32)
        nc.gpsimd.dma_start(bkt_sb[:], bkt_d)
        bvr_sb = consts.tile([P, HDC], F32)
        nc.gpsimd.dma_start(bvr_sb[:], bvr_d)
        mb_sb = consts.tile([P, ST_TILES], F32)
        nc.gpsimd.dma_start(mb_sb[:], mb_d)
        ones_sb = consts.tile([P, 64], DT)
        nc.vector.memset(ones_sb[:], 1.0)

        wk_sb = consts.tile([P, KO, HDC], DT)
        nc.gpsimd.dma_start(wk_sb[:], wk_d.rearrange("(ko p) h -> p ko h", p=P))
        wq_sb = consts.tile([P, KO, HDC], DT)
        nc.gpsimd.dma_start(wq_sb[:], wq_d.rearrange("(ko p) h -> p ko h", p=P))
        wv_sb = consts.tile([P, KO, HDC], DT)
        nc.gpsimd.dma_start(wv_sb[:], wv_d.rearrange("(ko p) h -> p ko h", p=P))

        xt_sb = consts.tile([P, KO, S], DT)
        xt_r = xt_d.rearrange("(ko p) s -> p ko s", p=P)
        for c in range(NQ):
            cs = slice(c * 512, (c + 1) * 512)
            for ko in range(KO):
                nc.sync.dma_start(xt_sb[:, ko, cs], xt_r[:, ko, cs])

        wo_sb = consts.tile([P, M2, H], DT)
        nc.gpsimd.dma_start(wo_sb[:], wo_d.rearrange("(m p) n -> p m n", p=P))

        qt_sb = qkv.tile([P, M2, S], DT)
        kt_sb = qkv.tile([P, M2, S], DT)
        v_sb = qkv.tile([P, ST_TILES, HDC], DT)
        ot_sb = qkv.tile([P, M2, S], DT)

        # ---- projection group emitters ----
        def proj_qk(w_sb, b_sb, out_sb, m, q):
            qs = slice(q * 512, (q + 1) * 512)
            ps = ps_proj.tile([P, 512], F32, tag="ps", name="ps_qk")
            for ko in range(KO):
                nc.tensor.matmul(
                    ps[:],
                    lhsT=w_sb[:, ko, m * P:(m + 1) * P],
                    rhs=xt_sb[:, ko, qs],
                    start=(ko == 0), stop=(ko == KO - 1),
                )
            nc.vector.tensor_add(
                out_sb[:, m, qs], ps[:],
                b_sb[:, m:m + 1].to_broadcast((P, 512)),
            )

        def proj_v(st):
            ps_full = ps_proj.tile([P, 512], F32, tag="ps", name="ps_v")
            ps = ps_full[:, :HDC]
            for ko in range(KO):
                nc.tensor.matmul(
                    ps[:],
                    lhsT=xt_sb[:, ko, st * P:(st + 1) * P],
                    rhs=wv_sb[:, ko, :],
                    start=(ko == 0), stop=(ko == KO - 1),
                )
            nc.vector.tensor_add(v_sb[:, st, :], ps[:], bvr_sb[:])

        def emit_y_tile(st, n):
            ss = slice(st * P, (st + 1) * P)
            ns = slice(n * 512, (n + 1) * 512)
            yp = ps_proj.tile([P, 512], F32, tag="ps", name="yp")
            for m in range(M2):
                nc.tensor.matmul(
                    yp[:],
                    lhsT=ot_sb[:, m, ss], rhs=wo_sb[:, m, ns],
                    start=(m == 0), stop=(m == M2 - 1),
                )
            y_sb = y_pool.tile([P, 512], F32, name="y_sb")
            nc.vector.tensor_copy(y_sb[:], yp[:])
            nc.sync.dma_start(y_d[ss, ns], y_sb[:])

        # ---- backlog of work drained through the pipeline ----
        # items: (deadline_iter, kind, args); kept sorted by deadline.
        backlog = []

        def add(deadline, kind, *args):
            backlog.append((deadline, kind, args))

        def run_item(kind, args):
            if kind == "qk":
                w_sb, b_sb, out_sb, m, q = args
                proj_qk(w_sb, b_sb, out_sb, m, q)
            elif kind == "v":
                proj_v(args[0])
            elif kind == "y":
                emit_y_tile(*args)

        def drain(i, budget):
            """Emit backlog items: all whose deadline is within 2 iters, then
            up to `budget` more."""
            backlog.sort(key=lambda t: t[0])
            n = 0
            while backlog:
                dl, kind, args = backlog[0]
                if dl <= i + 2 or n < budget:
                    backlog.pop(0)
                    run_item(kind, args)
                    n += 1
                else:
                    break

        # attention group order: m-outer so m=1 projections have slack
        seq = [(q, m, kt) for m in range(M2) for q in range(NQ)
               for kt in range(ST_TILES)]
        giter = {}  # (q, m) -> start iter
        for i, (q, m, kt) in enumerate(seq):
            if kt == 0:
                giter[(q, m)] = i

        # prefix: just enough for the pipeline to start
        proj_qk(wk_sb, bkt_sb, kt_sb, 0, 0)   # KT m0 chunk 0 (kt 0-3)
        proj_qk(wq_sb, bqt_sb, qt_sb, 0, 0)   # QT m0 q0
        proj_v(0)
        proj_v(1)

        # backlog deadlines
        for m in range(M2):
            for j in range(NQ):
                if (m, j) != (0, 0):
                    # KT chunk j needed by kt=4j of every group of this m
                    add(giter[(0, m)] + 4 * j if (m, j) != (0, 0) else 0,
                        "qk", wk_sb, bkt_sb, kt_sb, m, j)
                if (m, j) != (0, 0):
                    add(giter[(j, m)], "qk", wq_sb, bqt_sb, qt_sb, m, j)
        for st in range(2, ST_TILES):
            add(st, "v", st)

        # ---- flat attention pipeline ----
        def st_mms(q, m, kt):
            ks = slice(kt * P, (kt + 1) * P)
            qs = slice(q * 512, (q + 1) * 512)
            stp = ps_st.tile([P, 1024], F32, name="stp", tag="stp")
            nc.tensor.matmul(
                stp[:, 0:512],
                lhsT=kt_sb[0:64, m, ks], rhs=qt_sb[0:64, m, qs],
                start=True, stop=True,
            )
            nc.tensor.matmul(
                stp[:, 512:1024],
                lhsT=kt_sb[64:128, m, ks], rhs=qt_sb[64:128, m, qs],
                start=True, stop=True,
            )
            return stp

        ot_ps = d_ps = None
        stp_cur = st_mms(*seq[0])
        for i, (q, m, kt) in enumerate(seq):
            qs = slice(q * 512, (q + 1) * 512)
            hA, hB = 2 * m, 2 * m + 1
            if kt == 0:
                ot_ps = ps_ot.tile([P, 512], F32, name="ot_ps")
                d_ps = ps_d.tile([P, 512], F32, name="d_ps")
            pt = pt_pool.tile([P, 1024], DT, name="pt")
            nc.scalar.activation(
                pt[:], stp_cur[:],
                mybir.ActivationFunctionType.Exp,
                bias=mb_sb[:, kt:kt + 1],
                scale=SCALE,
            )
            if i + 1 < len(seq):
                stp_next = st_mms(*seq[i + 1])
            # O^T accumulation, col-packed pair (head A rows 0:64, B 64:128)
            nc.tensor.matmul(
                ot_ps[0:64, :],
                lhsT=v_sb[:, kt, hA * HD:(hA + 1) * HD], rhs=pt[:, 0:512],
                start=(kt == 0), stop=(kt == ST_TILES - 1),
            )
            nc.tensor.matmul(
                ot_ps[64:128, :],
                lhsT=v_sb[:, kt, hB * HD:(hB + 1) * HD], rhs=pt[:, 512:1024],
                start=(kt == 0), stop=(kt == ST_TILES - 1),
            )
            # denominators, replicated across 64 rows by 64 ones-columns
            nc.tensor.matmul(
                d_ps[0:64, :], lhsT=ones_sb[:, 0:64], rhs=pt[:, 0:512],
                start=(kt == 0), stop=(kt == ST_TILES - 1),
            )
            nc.tensor.matmul(
                d_ps[64:128, :], lhsT=ones_sb[:, 0:64], rhs=pt[:, 512:1024],
                start=(kt == 0), stop=(kt == ST_TILES - 1),
            )
            if kt == ST_TILES - 1:
                rec = sm_pool.tile([P, 512], F32, tag="rec", name="rec")
                nc.vector.reciprocal_approx_fast(rec[:], d_ps[:])
                nc.vector.tensor_mul(ot_sb[:, m, qs], ot_ps[:], rec[:])
                if m == M2 - 1:
                    idx = 0
                    for st in range(q * 4, q * 4 + 4):
                        for n in range(2):
                            add(i + 2 + 2 * idx, "y", st, n)
                            idx += 1
            drain(i, 1)
            stp_cur = stp_next
        drain(10 ** 9, 10 ** 9)

    nc.compile()
    return nc


def _get_built(dt_name="bfloat16"):
    if dt_name not in _BUILT:
        _BUILT[dt_name] = _build(dt_name)
    return _BUILT[dt_name]


def _prep_core_inputs(c, hidden_states, attention_mask, Wq, bq, Wk, bk, Wv, bv, Wo, bo,
                      np_dt):
    b, g = c // 4, c % 4
    hs = slice(g * HDC, (g + 1) * HDC)
    xt = np.ascontiguousarray(hidden_states[b].T).astype(np_dt)
    mb = np.where(attention_mask[b] == 0, np.float32(-30000.0), np.float32(0.0))
    return {
        "xt": xt,
        "wq": np.ascontiguousarray(Wq[:, hs]).astype(np_dt),
        "wk": np.ascontiguousarray(Wk[:, hs]).astype(np_dt),
        "wv": np.ascontiguousarray(Wv[:, hs]).astype(np_dt),
        "wo": np.ascontiguousarray(Wo[hs, :]).astype(np_dt),
        "bqt": np.ascontiguousarray(bq[hs].reshape(M2, P).T).astype(np.float32),
        "bkt": np.ascontiguousarray(bk[hs].reshape(M2, P).T).astype(np.float32),
        "bvr": np.tile(bv[hs].astype(np.float32), (P, 1)),
        "mb": np.ascontiguousarray(mb.astype(np.float32).reshape(ST_TILES, P).T),
    }


def kernel(hidden_states, attention_mask, Wq, bq, Wk, bk, Wv, bv, Wo, bo,
           _trace=False, _trace_kwargs=None):
    from concourse.bass_utils import run_bass_kernel_spmd

    hidden_states = np.asarray(hidden_states, np.float32)
    attention_mask = np.asarray(attention_mask)
    Wq, bq = np.asarray(Wq, np.float32), np.asarray(bq, np.float32)
    Wk, bk = np.asarray(Wk, np.float32), np.asarray(bk, np.float32)
    Wv, bv = np.asarray(Wv, np.float32), np.asarray(bv, np.float32)
    Wo, bo = np.asarray(Wo, np.float32), np.asarray(bo, np.float32)

    nc = _get_built()
    np_dt = ml_dtypes.bfloat16
    in_maps = [
        _prep_core_inputs(c, hidden_states, attention_mask,
                          Wq, bq, Wk, bk, Wv, bv, Wo, bo, np_dt)
        for c in range(8)
    ]
    kwargs = {}
    if _trace:
        kwargs["trace"] = True
        if _trace_kwargs:
            kwargs.update(_trace_kwargs)
    res = run_bass_kernel_spmd(nc, in_maps, core_ids=list(range(8)), **kwargs)
    out = np.empty((B, S, H), np.float32)
    for b in range(B):
        acc = res.results[4 * b]["y"].astype(np.float32).copy()
        for c in range(4 * b + 1, 4 * b + 4):
            acc += res.results[c]["y"]
        out[b] = acc + bo[None, :]
    if _trace:
        return out, res
    return out
